# revision 11
# baseline (speedup 1.0000x reference)
import sys
if "/opt/trn_rl_repo" not in sys.path:
    sys.path.insert(0, "/opt/trn_rl_repo")
import numpy as np
import concourse.bass as bass
import concourse.bacc as bacc
import concourse.mybir as mybir
import concourse.tile as tile
from concourse.bass_utils import run_bass_kernel_spmd

F32 = mybir.dt.float32
F32R = mybir.dt.float32r
AF = mybir.ActivationFunctionType
OP = mybir.AluOpType
AX = mybir.AxisListType

# problem dims (hardcoded)
V, E, H, A = 32000, 256, 512, 64
B, T_FULL, S = 32, 64, 128
NC = 8
BL = B // NC          # 4 batch rows per core
NG = 3 * H            # 1536 gate dims
VSL = V // NC         # 4000 vocab slice per core

_NC_CACHE = {}


def _build(with_bias, with_mask, with_fcb, T=T_FULL):
    nc = bacc.Bacc(None, num_devices=NC)
    dp = lambda name, shape, out=False: nc.declare_dram_parameter(name, list(shape), F32, isOutput=out)

    encT = dp("encT", [128, BL, 8, 128])      # encT[k,b,c,s] = enc[b,s,c*128+k]
    WihcT = dp("WihcT", [128, 8, NG])         # W_ih[:,E:].T chunks
    WhhT = dp("WhhT", [128, 4, NG])           # W_hh.T chunks
    WihET = dp("WihET", [128, 2, NG])         # W_ih[:,:E].T chunks
    Whr = dp("Whr", [128, 8, A])              # W_h chunks
    Wsr = dp("Wsr", [128, 4, A])              # W_s chunks
    vat = dp("vat", [128, BL * A])            # v_a tiled [s,(b,a)]
    embT = dp("embT", [128, 2, T, BL])        # embT[k,c,t,b]
    h0T = dp("h0T", [128, 4, BL])             # h0 transposed chunks
    h0b = dp("h0b", [BL, H])                  # h0 b-layout
    fcwT = dp("fcwT", [128, 4, VSL])          # fc_w slice .T chunks
    ident = dp("ident", [128, 128])
    ones4 = dp("ones4", [BL, 128])
    onescol = dp("onescol", [128, 1])
    onesrow = dp("onesrow", [1, 128])
    mask4 = dp("mask4", [BL, BL * A])         # delta mask for ws block-diag
    if with_mask:
        maskb = dp("maskb", [128, BL])        # additive score bias [s,b]
    if with_bias:
        biasrow = dp("biasrow", [1, 2048])    # [bih+bhh 0:1024 | bih_n | bhh_n]
    if with_fcb:
        fcb = dp("fcb", [1, VSL])

    logits_o = dp("logits_o", [B, T, VSL], out=True)
    attn_o = dp("attn_o", [BL, T, S], out=True)

    hT_loc = nc.dram_tensor("hT_loc", [4, 128, T * BL], F32)
    hT_all = nc.dram_tensor("hT_all", [NC, 4, 128, T * BL], F32)

    with tile.TileContext(nc) as tc:
        with tc.tile_pool(name="const", bufs=1) as cpool:
            # ---------------- persistent SBUF tiles ----------------
            encW_r = cpool.tile([128, BL, NG], F32R)     # enc @ WihC.T per b
            whT_r = cpool.tile([128, BL * A], F32R)      # Wh_T [s,(b,a)]
            whhT_r = cpool.tile([128, 4, NG], F32R)
            wihET_r = cpool.tile([128, 2, NG], F32R)
            wsr_r = cpool.tile([128, 4, A], F32R)
            vat_sb = cpool.tile([128, BL * A], F32)
            embT_r = cpool.tile([128, 2, T, BL], F32R)
            ident_r = cpool.tile([128, 128], F32R)
            ones4_r = cpool.tile([BL, 128], F32R)
            onescol_sb = cpool.tile([128, 1], F32)
            onesrow_sb = cpool.tile([1, 128], F32)
            onesrow_r = cpool.tile([1, 128], F32R)
            mask4_sb = cpool.tile([BL, BL * A], F32)
            h_hist = cpool.tile([128, 4, T + 1, BL], F32R)   # h_T chunks over time
            attnbuf = cpool.tile([128, T * BL], F32)
            bd = cpool.tile([128, BL, BL], F32R)             # block-diag attn
            h0b_sb = cpool.tile([BL, H], F32)
            if with_mask:
                maskb_sb = cpool.tile([128, BL], F32)
            if with_bias:
                biasrow_r = cpool.tile([1, 2048], F32R)
            if with_fcb:
                fcb_r = cpool.tile([1, VSL], F32R)

            # ---------------- load + round (chunked staging) ----------------
            with tc.tile_pool(name="stage", bufs=3) as spool:
                def load_round(dst_r, src_ap):
                    # stage one slab then round-copy to f32r
                    st = spool.tile(list(src_ap.shape), F32, tag="stg")
                    nc.gpsimd.dma_start(out=st[:], in_=src_ap)
                    nc.vector.tensor_copy(dst_r, st[:])

                for c in range(4):
                    load_round(whhT_r[:, c, :], WhhT[:, c, :])
                for c in range(2):
                    load_round(wihET_r[:, c, :], WihET[:, c, :])
                load_round(wsr_r[:].rearrange("k c a -> k (c a)"), Wsr[:].rearrange("k c a -> k (c a)"))
                load_round(embT_r[:].rearrange("k c t b -> k (c t b)"), embT[:].rearrange("k c t b -> k (c t b)"))
                load_round(ident_r[:], ident[:])
                load_round(ones4_r[:], ones4[:])
                nc.gpsimd.dma_start(out=onescol_sb[:], in_=onescol[:])
                load_round(onesrow_r[:], onesrow[:])
                nc.gpsimd.dma_start(out=vat_sb[:], in_=vat[:])
                nc.gpsimd.dma_start(out=onesrow_sb[:], in_=onesrow[:])
                nc.gpsimd.dma_start(out=mask4_sb[:], in_=mask4[:])
                nc.gpsimd.dma_start(out=h0b_sb[:], in_=h0b[:])
                if with_mask:
                    nc.gpsimd.dma_start(out=maskb_sb[:], in_=maskb[:])
                if with_bias:
                    load_round(biasrow_r[:], biasrow[:])
                if with_fcb:
                    load_round(fcb_r[:], fcb[:])
                h0st = spool.tile([128, 4, BL], F32, tag="h0")
                nc.gpsimd.dma_start(out=h0st[:], in_=h0T[:])
                nc.vector.tensor_copy(h_hist[:, :, 0, :], h0st[:])

                # ---------------- precompute encW + Wh_T ----------------
                with tc.tile_pool(name="pcpool", bufs=1) as pcpool, \
                     tc.tile_pool(name="pps", bufs=2, space="PSUM") as ppsp:
                    encT_r = pcpool.tile([128, BL, 8, 128], F32R)
                    wihcT_r = pcpool.tile([128, 8, NG], F32R)
                    whr_r = pcpool.tile([128, 8, A], F32R)
                    for b in range(BL):
                        load_round(encT_r[:, b, :, :].rearrange("k c s -> k (c s)"),
                                   encT[:, b, :, :].rearrange("k c s -> k (c s)"))
                    for c in range(8):
                        load_round(wihcT_r[:, c, :], WihcT[:, c, :])
                    load_round(whr_r[:].rearrange("k c a -> k (c a)"), Whr[:].rearrange("k c a -> k (c a)"))
                    for b in range(BL):
                        for nsl in range(3):
                            pps = ppsp.tile([128, 512], F32, tag="pcps")
                            for c in range(8):
                                nc.tensor.matmul(pps[:], encT_r[:, b, c, :],
                                                 wihcT_r[:, c, nsl * 512:(nsl + 1) * 512],
                                                 start=(c == 0), stop=(c == 7))
                            nc.vector.tensor_copy(encW_r[:, b, nsl * 512:(nsl + 1) * 512], pps[:])
                        ppw = ppsp.tile([128, A], F32, tag="pcwh")
                        for c in range(8):
                            nc.tensor.matmul(ppw[:], encT_r[:, b, c, :], whr_r[:, c, :],
                                             start=(c == 0), stop=(c == 7))
                        nc.vector.tensor_copy(whT_r[:, b * A:(b + 1) * A], ppw[:])

            # zero the bd off-diagonal once
            zb_pool_tile = cpool.tile([128, BL, BL], F32)
            nc.vector.memset(zb_pool_tile[:], 0.0)
            nc.vector.tensor_copy(bd[:], zb_pool_tile[:])

            # fcw pool outlives the recurrence (used by logits at the end)
            with tc.tile_pool(name="fcw", bufs=1) as fcwpool:
                fcw_r = fcwpool.tile([128, 4, VSL], F32R)
                with tc.tile_pool(name="fcwstage", bufs=2) as fstg:
                    for c in range(4):
                        st = fstg.tile([128, VSL], F32, tag="fcwstg")
                        nc.gpsimd.dma_start(out=st[:], in_=fcwT[:, c, :])
                        nc.vector.tensor_copy(fcw_r[:, c, :], st[:])

                # ---------------- recurrence ----------------
                with tc.tile_pool(name="step", bufs=2) as st_pool, \
                     tc.tile_pool(name="gps", bufs=1, space="PSUM") as gpsp, \
                     tc.tile_pool(name="mps", bufs=1, space="PSUM") as mpsp:
                    h_prev = h0b_sb
                    for t in range(T):
                        hT_cur = h_hist[:, :, t, :]   # [128, 4, BL]

                        # Ws = h @ W_s -> psum [BL, A] (in shared misc tile)
                        misc = mpsp.tile([128, 128], F32, tag="misc")
                        for c in range(4):
                            nc.tensor.matmul(misc[0:BL, 0:A], hT_cur[:, c, :], wsr_r[:, c, :],
                                             start=(c == 0), stop=(c == 3))
                        ws_sb = st_pool.tile([BL, A], F32, tag="ws")
                        nc.scalar.copy(out=ws_sb[:], in_=misc[0:BL, 0:A])
                        ws_bd = st_pool.tile([BL, BL * A], F32R, tag="wsbd")
                        ws_ap = ws_sb[:]
                        ws_bc = bass.AP(tensor=ws_ap.tensor, offset=ws_ap.offset,
                                        ap=[ws_ap.ap[0], [0, BL], list(ws_ap.ap[1])])
                        nc.vector.tensor_tensor(
                            out=ws_bd[:].rearrange("p (b a) -> p b a", b=BL),
                            in0=ws_bc,
                            in1=mask4_sb[:].rearrange("p (b a) -> p b a", b=BL), op=OP.mult)
                        # pre_T = ones4.T @ ws_bd + I.T @ whT  -> psum [128, 256]
                        pre = mpsp.tile([128, BL * A], F32, tag="pre")
                        nc.tensor.matmul(pre[:], ones4_r[:], ws_bd[:], start=True, stop=False)
                        nc.tensor.matmul(pre[:], ident_r[:], whT_r[:], start=False, stop=True)
                        tanh_t = st_pool.tile([128, BL * A], F32, tag="tanh")
                        nc.scalar.activation(tanh_t[:], pre[:], AF.Tanh)
                        vt = st_pool.tile([128, BL * A], F32, tag="vt")
                        nc.vector.tensor_tensor(out=vt[:], in0=tanh_t[:], in1=vat_sb[:], op=OP.mult)
                        score_t = st_pool.tile([128, BL], F32, tag="score")
                        nc.vector.tensor_reduce(out=score_t[:], in_=vt[:].rearrange("s (b a) -> s b a", b=BL),
                                                axis=AX.X, op=OP.add)
                        if with_mask:
                            nc.vector.tensor_tensor(out=score_t[:], in0=score_t[:], in1=maskb_sb[:], op=OP.add)
                        exp_t = st_pool.tile([128, BL], F32, tag="exp")
                        nc.scalar.activation(exp_t[:], score_t[:], AF.Exp)
                        # sums over s -> [1, BL]; reciprocal; broadcast to [128, BL]
                        misc2 = mpsp.tile([128, 128], F32, tag="misc2")
                        nc.tensor.matmul(misc2[0:1, 0:BL], onescol_sb[:], exp_t[:], start=True, stop=True)
                        inv = st_pool.tile([1, BL], F32, tag="inv")
                        nc.vector.reciprocal(out=inv[:], in_=misc2[0:1, 0:BL])
                        nc.tensor.matmul(misc2[:, 8:8 + BL], onesrow_sb[:], inv[:], start=True, stop=True)
                        nc.vector.tensor_tensor(out=attnbuf[:, t * BL:(t + 1) * BL], in0=exp_t[:],
                                                in1=misc2[:, 8:8 + BL], op=OP.mult)
                        bdflat = bd[:].rearrange("s b c -> s (b c)")
                        diag = bass.AP(tensor=bdflat.tensor, offset=bdflat.offset,
                                       ap=[bdflat.ap[0], [bdflat.ap[1][0] * (BL + 1), BL]])
                        nc.vector.tensor_copy(diag, attnbuf[:, t * BL:(t + 1) * BL])

                        # gates matmul: G [4, 2048] = 4 psum banks {r, z, n_gi, n_gh}
                        G = gpsp.tile([BL, 2048], F32, tag="G")
                        for j, srcs in enumerate([
                            ("h", "emb", "bd", "bias"),    # r-sum:  gate cols 0:512
                            ("h", "emb", "bd", "bias"),    # z-sum:  gate cols 512:1024
                            ("emb", "bd", "biasn_i"),      # n_gi:   gate cols 1024:1536
                            ("h", "biasn_h"),              # n_gh:   gate cols 1024:1536
                        ]):
                            lo = [0, 512, 1024, 1024][j]
                            mms = []
                            if "h" in srcs:
                                for c in range(4):
                                    mms.append((hT_cur[:, c, :], whhT_r[:, c, lo:lo + 512]))
                            if "emb" in srcs:
                                for c in range(2):
                                    mms.append((embT_r[:, c, t, :], wihET_r[:, c, lo:lo + 512]))
                            if "bd" in srcs:
                                for b in range(BL):
                                    mms.append((bd[:, b, :], encW_r[:, b, lo:lo + 512]))
                            if with_bias:
                                if "bias" in srcs:
                                    mms.append((ones4_r[0:1, 0:BL], biasrow_r[:, lo:lo + 512]))
                                if "biasn_i" in srcs:
                                    mms.append((ones4_r[0:1, 0:BL], biasrow_r[:, 1024:1536]))
                                if "biasn_h" in srcs:
                                    mms.append((ones4_r[0:1, 0:BL], biasrow_r[:, 1536:2048]))
                            for i, (lh, rh) in enumerate(mms):
                                nc.tensor.matmul(G[:, j * 512:(j + 1) * 512], lh, rh,
                                                 start=(i == 0), stop=(i == len(mms) - 1))

                        # gate nonlinearities (b-layout [4, 512])
                        r_sb = st_pool.tile([BL, H], F32, tag="r")
                        z_sb = st_pool.tile([BL, H], F32, tag="z")
                        n_sb = st_pool.tile([BL, H], F32, tag="n")
                        tmp = st_pool.tile([BL, H], F32, tag="tmp")
                        h_new = st_pool.tile([BL, H], F32, tag="hnew")
                        nc.scalar.activation(r_sb[:], G[:, 0:512], AF.Sigmoid)
                        nc.scalar.activation(z_sb[:], G[:, 512:1024], AF.Sigmoid)
                        nc.vector.tensor_tensor(out=tmp[:], in0=r_sb[:], in1=G[:, 1536:2048], op=OP.mult)
                        nc.vector.tensor_tensor(out=tmp[:], in0=tmp[:], in1=G[:, 1024:1536], op=OP.add)
                        nc.scalar.activation(n_sb[:], tmp[:], AF.Tanh)
                        nc.vector.tensor_tensor(out=tmp[:], in0=h_prev[:], in1=n_sb[:], op=OP.subtract)
                        nc.vector.tensor_tensor(out=tmp[:], in0=z_sb[:], in1=tmp[:], op=OP.mult)
                        nc.vector.tensor_tensor(out=h_new[:], in0=n_sb[:], in1=tmp[:], op=OP.add)
                        h_prev = h_new

                        # transpose h_new -> h_hist[:, :, t+1, :]
                        tps = mpsp.tile([128, 16], F32, tag="tps")
                        for c in range(4):
                            nc.tensor.transpose(tps[:, c * 4:c * 4 + 4], h_new[:, c * 128:(c + 1) * 128],
                                                ident_r[0:BL, 0:BL].bitcast(F32))
                        nc.vector.tensor_copy(h_hist[:, :, t + 1, :], tps[:].rearrange("s (c b) -> s c b", c=4))

                # ---------------- attn out + allgather + logits ----------------
                with tc.tile_pool(name="fin", bufs=2) as fpool, \
                     tc.tile_pool(name="fps", bufs=2, space="PSUM") as fpsp:
                    for half in range(T * BL // 128):
                        tp = fpsp.tile([128, 128], F32, tag="atp")
                        nc.tensor.transpose(tp[:], attnbuf[:, half * 128:(half + 1) * 128],
                                            ident_r[:].bitcast(F32))
                        asb = fpool.tile([128, 128], F32, tag="asb")
                        nc.vector.tensor_copy(asb[:], tp[:])
                        tchunk = 128 // BL
                        nc.gpsimd.dma_start(
                            out=attn_o[:, half * tchunk:(half + 1) * tchunk, :].rearrange("b t s -> t b s"),
                            in_=asb[:])

                    for c in range(4):
                        nc.gpsimd.dma_start(out=hT_loc[c, :, :],
                                            in_=h_hist[:, c, 1:T + 1, :].rearrange("s t b -> s (t b)").bitcast(F32))
                    nc.gpsimd.collective_compute(
                        "AllGather", OP.bypass,
                        replica_groups=[list(range(NC))],
                        ins=[hT_loc[:]], outs=[hT_all[:]])

                    hall = []
                    for c in range(4):
                        hall_c = fpool.tile([128, NC * T * BL], F32R, tag=f"hall{c}", bufs=1)
                        hall.append(hall_c)
                    for c in range(4):
                        stg = fpool.tile([128, NC, T * BL], F32, tag="hallstg")
                        nc.gpsimd.dma_start(out=stg[:], in_=hT_all[:, c, :, :].rearrange("r s m -> s r m"))
                        nc.vector.tensor_copy(hall[c][:], stg[:].rearrange("s r m -> s (r m)"))

                    n_m = NC * T * BL // 128
                    tb_per_m = 128 // BL
                    for m in range(n_m):
                        for nsl in range(8):
                            lps = fpsp.tile([128, 512], F32, tag="lps")
                            for c in range(4):
                                nc.tensor.matmul(lps[:, 0:500], hall[c][:, m * 128:(m + 1) * 128],
                                                 fcw_r[:, c, nsl * 500:(nsl + 1) * 500],
                                                 start=(c == 0), stop=(c == 3 and not with_fcb))
                            if with_fcb:
                                nc.tensor.matmul(lps[:, 0:500], onesrow_r[:],
                                                 fcb_r[:, nsl * 500:(nsl + 1) * 500],
                                                 start=False, stop=True)
                            lsb = fpool.tile([128, 500], F32, tag="lsb")
                            nc.vector.tensor_copy(lsb[:], lps[:, 0:500])
                            r0 = m // (T // tb_per_m)
                            t0 = (m % (T // tb_per_m)) * tb_per_m
                            nc.gpsimd.dma_start(
                                out=logits_o[BL * r0:BL * r0 + BL, t0:t0 + tb_per_m,
                                             nsl * 500:(nsl + 1) * 500].rearrange("b t v -> t b v"),
                                in_=lsb[:])

    nc.finalize()
    return nc


def _prep_inputs(trg_input, encoder_outputs, encoder_mask, hidden, emb_table,
                 W_h, W_s, v_a, W_ih, W_hh, b_ih, b_hh, fc_w, fc_b, T=T_FULL):
    f = np.float32
    trg = np.asarray(trg_input)
    enc = np.asarray(encoder_outputs, f)
    mask = np.asarray(encoder_mask)
    h0 = np.asarray(hidden, f)[-1]                     # [B, H]
    embt = np.asarray(emb_table, f)
    W_h = np.asarray(W_h, f); W_s = np.asarray(W_s, f); v_a = np.asarray(v_a, f)
    W_ih = np.asarray(W_ih, f); W_hh = np.asarray(W_hh, f)
    b_ih = np.asarray(b_ih, f); b_hh = np.asarray(b_hh, f)
    fc_w = np.asarray(fc_w, f); fc_b = np.asarray(fc_b, f)

    with_bias = bool(np.any(b_ih) or np.any(b_hh))
    with_mask = bool(np.any(np.asarray(mask) == 0))
    with_fcb = bool(np.any(fc_b))

    emb = embt[trg][:, :T, :]                          # [B, T, E]
    WihcT = np.ascontiguousarray(W_ih[:, E:].T.reshape(8, 128, NG).transpose(1, 0, 2))
    WhhT = np.ascontiguousarray(W_hh.T.reshape(4, 128, NG).transpose(1, 0, 2))
    WihET = np.ascontiguousarray(W_ih[:, :E].T.reshape(2, 128, NG).transpose(1, 0, 2))
    Whr = np.ascontiguousarray(W_h.reshape(8, 128, A).transpose(1, 0, 2))
    Wsr = np.ascontiguousarray(W_s.reshape(4, 128, A).transpose(1, 0, 2))
    vat = np.tile(v_a[None, :], (128, BL)).astype(f)
    identm = np.eye(128, dtype=f)
    ones4 = np.ones((BL, 128), f)
    onescol = np.ones((128, 1), f)
    onesrow = np.ones((1, 128), f)
    mask4 = np.zeros((BL, BL * A), f)
    for b in range(BL):
        mask4[b, b * A:(b + 1) * A] = 1.0
    biasrow = np.concatenate([b_ih[:2 * H] + b_hh[:2 * H], b_ih[2 * H:], b_hh[2 * H:]])[None, :].astype(f)

    in_maps = []
    for cc in range(NC):
        bs = slice(BL * cc, BL * cc + BL)
        enc_loc = enc[bs]
        encTl = np.ascontiguousarray(enc_loc.transpose(0, 2, 1).reshape(BL, 8, 128, S).transpose(2, 0, 1, 3))
        emb_loc = emb[bs]
        embTl = np.ascontiguousarray(emb_loc.transpose(2, 1, 0).reshape(2, 128, T, BL).transpose(1, 0, 2, 3))
        h0T = np.ascontiguousarray(h0[bs].T.reshape(4, 128, BL).transpose(1, 0, 2))
        fcwT = np.ascontiguousarray(fc_w[VSL * cc:VSL * (cc + 1), :].T.reshape(4, 128, VSL).transpose(1, 0, 2))
        m = {
            "encT": encTl, "WihcT": WihcT, "WhhT": WhhT, "WihET": WihET,
            "Whr": Whr, "Wsr": Wsr, "vat": vat, "embT": embTl, "h0T": h0T,
            "h0b": np.ascontiguousarray(h0[bs]),
            "fcwT": fcwT, "ident": identm, "ones4": ones4, "onescol": onescol,
            "onesrow": onesrow, "mask4": mask4,
        }
        if with_mask:
            mb = np.where(mask[bs] == 0, np.float32(-1e9), np.float32(0.0))
            m["maskb"] = np.ascontiguousarray(mb.T).astype(f)
        if with_bias:
            m["biasrow"] = biasrow
        if with_fcb:
            m["fcb"] = np.ascontiguousarray(fc_b[None, VSL * cc:VSL * (cc + 1)])
        in_maps.append(m)
    return in_maps, (with_bias, with_mask, with_fcb)


def kernel(**inputs):
    in_maps, flags, = _prep_inputs(**inputs)
    if flags not in _NC_CACHE:
        _NC_CACHE[flags] = _build(*flags)
    nc = _NC_CACHE[flags]
    res = run_bass_kernel_spmd(nc, in_maps, core_ids=list(range(NC))).results
    logits = np.concatenate([res[c]["logits_o"] for c in range(NC)], axis=2)
    attn = np.concatenate([res[c]["attn_o"] for c in range(NC)], axis=0)
    return logits, attn


# revision 12
# speedup vs baseline: 1.2412x; 1.2412x over previous
import sys
if "/opt/trn_rl_repo" not in sys.path:
    sys.path.insert(0, "/opt/trn_rl_repo")
import numpy as np
import concourse.bass as bass
import concourse.bacc as bacc
import concourse.mybir as mybir
import concourse.tile as tile
from concourse.bass_utils import run_bass_kernel_spmd
from concourse import bass2jax as _b2j

F32 = mybir.dt.float32
F32R = mybir.dt.float32r
AF = mybir.ActivationFunctionType
OP = mybir.AluOpType
AX = mybir.AxisListType

# problem dims (hardcoded)
V, E, H, A = 32000, 256, 512, 64
B, T_FULL, S = 32, 64, 128
NC = 8
BL = B // NC          # 4 batch rows per core
NG = 3 * H            # 1536 gate dims
VSL = V // NC         # 4000 vocab slice per core

_NC_CACHE = {}


def _build(with_bias, with_mask, with_fcb, T=T_FULL):
    nc = bacc.Bacc(None, num_devices=NC)
    dp = lambda name, shape, out=False: nc.declare_dram_parameter(name, list(shape), F32, isOutput=out)

    encT = dp("encT", [128, BL, 8, 128])      # encT[k,b,c,s] = enc[b,s,c*128+k]
    WihcT = dp("WihcT", [128, 8, NG])         # W_ih[:,E:].T chunks
    WhhT = dp("WhhT", [128, 4, NG])           # W_hh.T chunks
    WihET = dp("WihET", [128, 2, NG])         # W_ih[:,:E].T chunks
    Whr = dp("Whr", [128, 8, A])              # W_h chunks
    Wsr = dp("Wsr", [128, 4, A])              # W_s chunks
    vat = dp("vat", [128, BL * A])            # v_a tiled [s,(b,a)]
    embT = dp("embT", [128, 2, T, BL])        # embT[k,c,t,b]
    h0T = dp("h0T", [128, 4, BL])             # h0 transposed chunks
    h0b = dp("h0b", [BL, H])                  # h0 b-layout
    fcwT = dp("fcwT", [128, 4, VSL])          # fc_w slice .T chunks
    ident = dp("ident", [128, 128])
    ones4 = dp("ones4", [BL, 128])
    onescol = dp("onescol", [128, 1])
    onesrow = dp("onesrow", [1, 128])
    mask4 = dp("mask4", [BL, BL * A])         # delta mask for ws block-diag
    if with_mask:
        maskb = dp("maskb", [128, BL])        # additive score bias [s,b]
    if with_bias:
        biasrow = dp("biasrow", [1, 2048])    # [bih+bhh 0:1024 | bih_n | bhh_n]
    if with_fcb:
        fcb = dp("fcb", [1, VSL])

    logits_o = dp("logits_o", [B, T, VSL], out=True)
    attn_o = dp("attn_o", [BL, T, S], out=True)

    hT_loc = nc.dram_tensor("hT_loc", [4, 128, T * BL], F32)
    hT_all = nc.dram_tensor("hT_all", [NC, 4, 128, T * BL], F32)

    with tile.TileContext(nc) as tc:
        with tc.tile_pool(name="const", bufs=1) as cpool:
            # ---------------- persistent SBUF tiles ----------------
            encW_r = cpool.tile([128, BL, NG], F32R)     # enc @ WihC.T per b
            whT_r = cpool.tile([128, BL * A], F32R)      # Wh_T [s,(b,a)]
            whhT_r = cpool.tile([128, 4, NG], F32R)
            wihET_r = cpool.tile([128, 2, NG], F32R)
            wsr_r = cpool.tile([128, 4, A], F32R)
            vat_sb = cpool.tile([128, BL * A], F32)
            embT_r = cpool.tile([128, 2, T, BL], F32R)
            ident_r = cpool.tile([128, 128], F32R)
            ones4_r = cpool.tile([BL, 128], F32R)
            onescol_sb = cpool.tile([128, 1], F32)
            onesrow_sb = cpool.tile([1, 128], F32)
            onesrow_r = cpool.tile([1, 128], F32R)
            mask4_sb = cpool.tile([BL, BL * A], F32)
            h_hist = cpool.tile([128, 4, T + 1, BL], F32R)   # h_T chunks over time
            attnbuf = cpool.tile([128, T * BL], F32)
            bd = cpool.tile([128, BL, BL], F32R)             # block-diag attn
            h0b_sb = cpool.tile([BL, H], F32)
            if with_mask:
                maskb_sb = cpool.tile([128, BL], F32)
            if with_bias:
                biasrow_r = cpool.tile([1, 2048], F32R)
            if with_fcb:
                fcb_r = cpool.tile([1, VSL], F32R)

            # ---------------- load + round (chunked staging) ----------------
            with tc.tile_pool(name="stage", bufs=3) as spool:
                def load_round(dst_r, src_ap):
                    # stage one slab then round-copy to f32r
                    st = spool.tile(list(src_ap.shape), F32, tag="stg")
                    nc.gpsimd.dma_start(out=st[:], in_=src_ap)
                    nc.vector.tensor_copy(dst_r, st[:])

                for c in range(4):
                    load_round(whhT_r[:, c, :], WhhT[:, c, :])
                for c in range(2):
                    load_round(wihET_r[:, c, :], WihET[:, c, :])
                load_round(wsr_r[:].rearrange("k c a -> k (c a)"), Wsr[:].rearrange("k c a -> k (c a)"))
                load_round(embT_r[:].rearrange("k c t b -> k (c t b)"), embT[:].rearrange("k c t b -> k (c t b)"))
                load_round(ident_r[:], ident[:])
                load_round(ones4_r[:], ones4[:])
                nc.gpsimd.dma_start(out=onescol_sb[:], in_=onescol[:])
                load_round(onesrow_r[:], onesrow[:])
                nc.gpsimd.dma_start(out=vat_sb[:], in_=vat[:])
                nc.gpsimd.dma_start(out=onesrow_sb[:], in_=onesrow[:])
                nc.gpsimd.dma_start(out=mask4_sb[:], in_=mask4[:])
                nc.gpsimd.dma_start(out=h0b_sb[:], in_=h0b[:])
                if with_mask:
                    nc.gpsimd.dma_start(out=maskb_sb[:], in_=maskb[:])
                if with_bias:
                    load_round(biasrow_r[:], biasrow[:])
                if with_fcb:
                    load_round(fcb_r[:], fcb[:])
                h0st = spool.tile([128, 4, BL], F32, tag="h0")
                nc.gpsimd.dma_start(out=h0st[:], in_=h0T[:])
                nc.vector.tensor_copy(h_hist[:, :, 0, :], h0st[:])

                # ---------------- precompute encW + Wh_T ----------------
                with tc.tile_pool(name="pcpool", bufs=1) as pcpool, \
                     tc.tile_pool(name="pps", bufs=2, space="PSUM") as ppsp:
                    encT_r = pcpool.tile([128, BL, 8, 128], F32R)
                    wihcT_r = pcpool.tile([128, 8, NG], F32R)
                    whr_r = pcpool.tile([128, 8, A], F32R)
                    for b in range(BL):
                        load_round(encT_r[:, b, :, :].rearrange("k c s -> k (c s)"),
                                   encT[:, b, :, :].rearrange("k c s -> k (c s)"))
                    for c in range(8):
                        load_round(wihcT_r[:, c, :], WihcT[:, c, :])
                    load_round(whr_r[:].rearrange("k c a -> k (c a)"), Whr[:].rearrange("k c a -> k (c a)"))
                    for b in range(BL):
                        for nsl in range(3):
                            pps = ppsp.tile([128, 512], F32, tag="pcps")
                            for c in range(8):
                                nc.tensor.matmul(pps[:], encT_r[:, b, c, :],
                                                 wihcT_r[:, c, nsl * 512:(nsl + 1) * 512],
                                                 start=(c == 0), stop=(c == 7))
                            nc.vector.tensor_copy(encW_r[:, b, nsl * 512:(nsl + 1) * 512], pps[:])
                        ppw = ppsp.tile([128, A], F32, tag="pcwh")
                        for c in range(8):
                            nc.tensor.matmul(ppw[:], encT_r[:, b, c, :], whr_r[:, c, :],
                                             start=(c == 0), stop=(c == 7))
                        nc.vector.tensor_copy(whT_r[:, b * A:(b + 1) * A], ppw[:])

            # zero the bd off-diagonal once
            zb_pool_tile = cpool.tile([128, BL, BL], F32)
            nc.vector.memset(zb_pool_tile[:], 0.0)
            nc.vector.tensor_copy(bd[:], zb_pool_tile[:])

            # fcw pool outlives the recurrence (used by logits at the end)
            with tc.tile_pool(name="fcw", bufs=1) as fcwpool:
                fcw_r = fcwpool.tile([128, 4, VSL], F32R)
                with tc.tile_pool(name="fcwstage", bufs=2) as fstg:
                    for c in range(4):
                        st = fstg.tile([128, VSL], F32, tag="fcwstg")
                        nc.gpsimd.dma_start(out=st[:], in_=fcwT[:, c, :])
                        nc.vector.tensor_copy(fcw_r[:, c, :], st[:])

                # ---------------- recurrence ----------------
                with tc.tile_pool(name="step", bufs=2) as st_pool, \
                     tc.tile_pool(name="gps", bufs=1, space="PSUM") as gpsp, \
                     tc.tile_pool(name="mps", bufs=1, space="PSUM") as mpsp:
                    h_prev = h0b_sb
                    for t in range(T):
                        hT_cur = h_hist[:, :, t, :]   # [128, 4, BL]

                        # Ws = h @ W_s -> psum [BL, A] (in shared misc tile)
                        misc = mpsp.tile([128, 128], F32, tag="misc")
                        for c in range(4):
                            nc.tensor.matmul(misc[0:BL, 0:A], hT_cur[:, c, :], wsr_r[:, c, :],
                                             start=(c == 0), stop=(c == 3))
                        ws_sb = st_pool.tile([BL, A], F32, tag="ws")
                        nc.scalar.copy(out=ws_sb[:], in_=misc[0:BL, 0:A])
                        ws_bd = st_pool.tile([BL, BL * A], F32R, tag="wsbd")
                        ws_ap = ws_sb[:]
                        ws_bc = bass.AP(tensor=ws_ap.tensor, offset=ws_ap.offset,
                                        ap=[ws_ap.ap[0], [0, BL], list(ws_ap.ap[1])])
                        nc.vector.tensor_tensor(
                            out=ws_bd[:].rearrange("p (b a) -> p b a", b=BL),
                            in0=ws_bc,
                            in1=mask4_sb[:].rearrange("p (b a) -> p b a", b=BL), op=OP.mult)
                        # pre_T = ones4.T @ ws_bd + I.T @ whT  -> psum [128, 256]
                        pre = mpsp.tile([128, BL * A], F32, tag="pre")
                        nc.tensor.matmul(pre[:], ones4_r[:], ws_bd[:], start=True, stop=False)
                        nc.tensor.matmul(pre[:], ident_r[:], whT_r[:], start=False, stop=True)
                        tanh_t = st_pool.tile([128, BL * A], F32, tag="tanh")
                        nc.scalar.activation(tanh_t[:], pre[:], AF.Tanh)
                        vt = st_pool.tile([128, BL * A], F32, tag="vt")
                        nc.vector.tensor_tensor(out=vt[:], in0=tanh_t[:], in1=vat_sb[:], op=OP.mult)
                        score_t = st_pool.tile([128, BL], F32, tag="score")
                        nc.vector.tensor_reduce(out=score_t[:], in_=vt[:].rearrange("s (b a) -> s b a", b=BL),
                                                axis=AX.X, op=OP.add)
                        if with_mask:
                            nc.vector.tensor_tensor(out=score_t[:], in0=score_t[:], in1=maskb_sb[:], op=OP.add)
                        exp_t = st_pool.tile([128, BL], F32, tag="exp")
                        nc.scalar.activation(exp_t[:], score_t[:], AF.Exp)
                        # sums over s -> [1, BL]; reciprocal; broadcast to [128, BL]
                        misc2 = mpsp.tile([128, 128], F32, tag="misc2")
                        nc.tensor.matmul(misc2[0:1, 0:BL], onescol_sb[:], exp_t[:], start=True, stop=True)
                        inv = st_pool.tile([1, BL], F32, tag="inv")
                        nc.vector.reciprocal(out=inv[:], in_=misc2[0:1, 0:BL])
                        nc.tensor.matmul(misc2[:, 8:8 + BL], onesrow_sb[:], inv[:], start=True, stop=True)
                        nc.vector.tensor_tensor(out=attnbuf[:, t * BL:(t + 1) * BL], in0=exp_t[:],
                                                in1=misc2[:, 8:8 + BL], op=OP.mult)
                        bdflat = bd[:].rearrange("s b c -> s (b c)")
                        diag = bass.AP(tensor=bdflat.tensor, offset=bdflat.offset,
                                       ap=[bdflat.ap[0], [bdflat.ap[1][0] * (BL + 1), BL]])
                        nc.vector.tensor_copy(diag, attnbuf[:, t * BL:(t + 1) * BL])

                        # gates matmul: G [4, 2048] = 4 psum banks {r, z, n_gi, n_gh}
                        G = gpsp.tile([BL, 2048], F32, tag="G")
                        for j, srcs in enumerate([
                            ("h", "emb", "bd", "bias"),    # r-sum:  gate cols 0:512
                            ("h", "emb", "bd", "bias"),    # z-sum:  gate cols 512:1024
                            ("emb", "bd", "biasn_i"),      # n_gi:   gate cols 1024:1536
                            ("h", "biasn_h"),              # n_gh:   gate cols 1024:1536
                        ]):
                            lo = [0, 512, 1024, 1024][j]
                            mms = []
                            if "h" in srcs:
                                for c in range(4):
                                    mms.append((hT_cur[:, c, :], whhT_r[:, c, lo:lo + 512]))
                            if "emb" in srcs:
                                for c in range(2):
                                    mms.append((embT_r[:, c, t, :], wihET_r[:, c, lo:lo + 512]))
                            if "bd" in srcs:
                                for b in range(BL):
                                    mms.append((bd[:, b, :], encW_r[:, b, lo:lo + 512]))
                            if with_bias:
                                if "bias" in srcs:
                                    mms.append((ones4_r[0:1, 0:BL], biasrow_r[:, lo:lo + 512]))
                                if "biasn_i" in srcs:
                                    mms.append((ones4_r[0:1, 0:BL], biasrow_r[:, 1024:1536]))
                                if "biasn_h" in srcs:
                                    mms.append((ones4_r[0:1, 0:BL], biasrow_r[:, 1536:2048]))
                            for i, (lh, rh) in enumerate(mms):
                                nc.tensor.matmul(G[:, j * 512:(j + 1) * 512], lh, rh,
                                                 start=(i == 0), stop=(i == len(mms) - 1))

                        # gate nonlinearities (b-layout [4, 512])
                        r_sb = st_pool.tile([BL, H], F32, tag="r")
                        z_sb = st_pool.tile([BL, H], F32, tag="z")
                        n_sb = st_pool.tile([BL, H], F32, tag="n")
                        tmp = st_pool.tile([BL, H], F32, tag="tmp")
                        h_new = st_pool.tile([BL, H], F32, tag="hnew")
                        nc.scalar.activation(r_sb[:], G[:, 0:512], AF.Sigmoid)
                        nc.scalar.activation(z_sb[:], G[:, 512:1024], AF.Sigmoid)
                        nc.vector.tensor_tensor(out=tmp[:], in0=r_sb[:], in1=G[:, 1536:2048], op=OP.mult)
                        nc.vector.tensor_tensor(out=tmp[:], in0=tmp[:], in1=G[:, 1024:1536], op=OP.add)
                        nc.scalar.activation(n_sb[:], tmp[:], AF.Tanh)
                        nc.vector.tensor_tensor(out=tmp[:], in0=h_prev[:], in1=n_sb[:], op=OP.subtract)
                        nc.vector.tensor_tensor(out=tmp[:], in0=z_sb[:], in1=tmp[:], op=OP.mult)
                        nc.vector.tensor_tensor(out=h_new[:], in0=n_sb[:], in1=tmp[:], op=OP.add)
                        h_prev = h_new

                        # transpose h_new -> h_hist[:, :, t+1, :]
                        tps = mpsp.tile([128, 16], F32, tag="tps")
                        for c in range(4):
                            nc.tensor.transpose(tps[:, c * 4:c * 4 + 4], h_new[:, c * 128:(c + 1) * 128],
                                                ident_r[0:BL, 0:BL].bitcast(F32))
                        nc.vector.tensor_copy(h_hist[:, :, t + 1, :], tps[:].rearrange("s (c b) -> s c b", c=4))

                # ---------------- attn out + allgather + logits ----------------
                with tc.tile_pool(name="fin", bufs=2) as fpool, \
                     tc.tile_pool(name="fps", bufs=2, space="PSUM") as fpsp:
                    for half in range(T * BL // 128):
                        tp = fpsp.tile([128, 128], F32, tag="atp")
                        nc.tensor.transpose(tp[:], attnbuf[:, half * 128:(half + 1) * 128],
                                            ident_r[:].bitcast(F32))
                        asb = fpool.tile([128, 128], F32, tag="asb")
                        nc.vector.tensor_copy(asb[:], tp[:])
                        tchunk = 128 // BL
                        nc.gpsimd.dma_start(
                            out=attn_o[:, half * tchunk:(half + 1) * tchunk, :].rearrange("b t s -> t b s"),
                            in_=asb[:])

                    for c in range(4):
                        nc.gpsimd.dma_start(out=hT_loc[c, :, :],
                                            in_=h_hist[:, c, 1:T + 1, :].rearrange("s t b -> s (t b)").bitcast(F32))
                    nc.gpsimd.collective_compute(
                        "AllGather", OP.bypass,
                        replica_groups=[list(range(NC))],
                        ins=[hT_loc[:]], outs=[hT_all[:]])

                    hall = []
                    for c in range(4):
                        hall_c = fpool.tile([128, NC * T * BL], F32R, tag=f"hall{c}", bufs=1)
                        hall.append(hall_c)
                    for c in range(4):
                        stg = fpool.tile([128, NC, T * BL], F32, tag="hallstg")
                        nc.gpsimd.dma_start(out=stg[:], in_=hT_all[:, c, :, :].rearrange("r s m -> s r m"))
                        nc.vector.tensor_copy(hall[c][:], stg[:].rearrange("s r m -> s (r m)"))

                    n_m = NC * T * BL // 128
                    tb_per_m = 128 // BL
                    for m in range(n_m):
                        for nsl in range(8):
                            lps = fpsp.tile([128, 512], F32, tag="lps")
                            for c in range(4):
                                nc.tensor.matmul(lps[:, 0:500], hall[c][:, m * 128:(m + 1) * 128],
                                                 fcw_r[:, c, nsl * 500:(nsl + 1) * 500],
                                                 start=(c == 0), stop=(c == 3 and not with_fcb))
                            if with_fcb:
                                nc.tensor.matmul(lps[:, 0:500], onesrow_r[:],
                                                 fcb_r[:, nsl * 500:(nsl + 1) * 500],
                                                 start=False, stop=True)
                            lsb = fpool.tile([128, 500], F32, tag="lsb")
                            nc.vector.tensor_copy(lsb[:], lps[:, 0:500])
                            r0 = m // (T // tb_per_m)
                            t0 = (m % (T // tb_per_m)) * tb_per_m
                            nc.gpsimd.dma_start(
                                out=logits_o[BL * r0:BL * r0 + BL, t0:t0 + tb_per_m,
                                             nsl * 500:(nsl + 1) * 500].rearrange("b t v -> t b v"),
                                in_=lsb[:])

    nc.finalize()
    return nc


def _prep_inputs(trg_input, encoder_outputs, encoder_mask, hidden, emb_table,
                 W_h, W_s, v_a, W_ih, W_hh, b_ih, b_hh, fc_w, fc_b, T=T_FULL):
    f = np.float32
    trg = np.asarray(trg_input)
    enc = np.asarray(encoder_outputs, f)
    mask = np.asarray(encoder_mask)
    h0 = np.asarray(hidden, f)[-1]                     # [B, H]
    embt = np.asarray(emb_table, f)
    W_h = np.asarray(W_h, f); W_s = np.asarray(W_s, f); v_a = np.asarray(v_a, f)
    W_ih = np.asarray(W_ih, f); W_hh = np.asarray(W_hh, f)
    b_ih = np.asarray(b_ih, f); b_hh = np.asarray(b_hh, f)
    fc_w = np.asarray(fc_w, f); fc_b = np.asarray(fc_b, f)

    with_bias = bool(np.any(b_ih) or np.any(b_hh))
    with_mask = bool(np.any(np.asarray(mask) == 0))
    with_fcb = bool(np.any(fc_b))

    emb = embt[trg][:, :T, :]                          # [B, T, E]
    WihcT = np.ascontiguousarray(W_ih[:, E:].T.reshape(8, 128, NG).transpose(1, 0, 2))
    WhhT = np.ascontiguousarray(W_hh.T.reshape(4, 128, NG).transpose(1, 0, 2))
    WihET = np.ascontiguousarray(W_ih[:, :E].T.reshape(2, 128, NG).transpose(1, 0, 2))
    Whr = np.ascontiguousarray(W_h.reshape(8, 128, A).transpose(1, 0, 2))
    Wsr = np.ascontiguousarray(W_s.reshape(4, 128, A).transpose(1, 0, 2))
    vat = np.tile(v_a[None, :], (128, BL)).astype(f)
    identm = np.eye(128, dtype=f)
    ones4 = np.ones((BL, 128), f)
    onescol = np.ones((128, 1), f)
    onesrow = np.ones((1, 128), f)
    mask4 = np.zeros((BL, BL * A), f)
    for b in range(BL):
        mask4[b, b * A:(b + 1) * A] = 1.0
    biasrow = np.concatenate([b_ih[:2 * H] + b_hh[:2 * H], b_ih[2 * H:], b_hh[2 * H:]])[None, :].astype(f)

    in_maps = []
    for cc in range(NC):
        bs = slice(BL * cc, BL * cc + BL)
        enc_loc = enc[bs]
        encTl = np.ascontiguousarray(enc_loc.transpose(0, 2, 1).reshape(BL, 8, 128, S).transpose(2, 0, 1, 3))
        emb_loc = emb[bs]
        embTl = np.ascontiguousarray(emb_loc.transpose(2, 1, 0).reshape(2, 128, T, BL).transpose(1, 0, 2, 3))
        h0T = np.ascontiguousarray(h0[bs].T.reshape(4, 128, BL).transpose(1, 0, 2))
        fcwT = np.ascontiguousarray(fc_w[VSL * cc:VSL * (cc + 1), :].T.reshape(4, 128, VSL).transpose(1, 0, 2))
        m = {
            "encT": encTl, "WihcT": WihcT, "WhhT": WhhT, "WihET": WihET,
            "Whr": Whr, "Wsr": Wsr, "vat": vat, "embT": embTl, "h0T": h0T,
            "h0b": np.ascontiguousarray(h0[bs]),
            "fcwT": fcwT, "ident": identm, "ones4": ones4, "onescol": onescol,
            "onesrow": onesrow, "mask4": mask4,
        }
        if with_mask:
            mb = np.where(mask[bs] == 0, np.float32(-1e9), np.float32(0.0))
            m["maskb"] = np.ascontiguousarray(mb.T).astype(f)
        if with_bias:
            m["biasrow"] = biasrow
        if with_fcb:
            m["fcb"] = np.ascontiguousarray(fc_b[None, VSL * cc:VSL * (cc + 1)])
        in_maps.append(m)
    return in_maps, (with_bias, with_mask, with_fcb)


def _make_runner(nc):
    """Persistent jitted SPMD executor for a finalized Bass module (adapted
    from bass2jax.run_bass_via_pjrt, with the jit + zero output params cached
    across calls instead of rebuilt per call)."""
    import jax
    import concourse.mybir as mb
    from jax.experimental.shard_map import shard_map
    from jax.sharding import Mesh, PartitionSpec

    _b2j.install_neuronx_cc_hook()
    partition_name = nc.partition_id_tensor.name if nc.partition_id_tensor else None
    in_names, out_names, out_avals, zero_outs = [], [], [], []
    for alloc in nc.main_func.allocations:
        if not isinstance(mb.MemoryLocationSet, type) or not isinstance(alloc, mb.MemoryLocationSet):
            continue
        name = alloc.memorylocations[0].name
        if alloc.kind == "ExternalInput":
            if name != partition_name:
                in_names.append(name)
        elif alloc.kind == "ExternalOutput":
            shape = tuple(alloc.tensor_shape)
            dtype = mb.dt.np(alloc.dtype)
            out_names.append(name)
            out_avals.append(jax.core.ShapedArray(shape, dtype))
            zero_outs.append(np.zeros(shape, dtype))
    n_params = len(in_names)
    all_in_names = list(in_names) + list(out_names)
    if partition_name is not None:
        all_in_names.append(partition_name)

    def _body(*args):
        operands = list(args)
        if partition_name is not None:
            operands.append(_b2j.partition_id_tensor())
        outs = _b2j._bass_exec_p.bind(
            *operands,
            out_avals=tuple(out_avals),
            in_names=tuple(all_in_names),
            out_names=tuple(out_names),
            lowering_input_output_aliases=(),
            sim_require_finite=True,
            sim_require_nnan=True,
            nc=nc,
        )
        return tuple(outs)

    devices = jax.devices()[:NC]
    mesh = Mesh(np.asarray(devices), ("core",))
    in_specs = (PartitionSpec("core"),) * (n_params + len(out_names))
    out_specs = (PartitionSpec("core"),) * len(out_names)
    jitted = jax.jit(shard_map(_body, mesh=mesh, in_specs=in_specs,
                               out_specs=out_specs, check_rep=False),
                     keep_unused=True)
    # zero "output" params live on device once (no donation -> reusable)
    zeros_dev = [jax.device_put(
        np.concatenate([z[None]] * NC, axis=0).reshape(NC * z.shape[0], *z.shape[1:]),
        jax.sharding.NamedSharding(mesh, PartitionSpec("core")))
        for z in zero_outs]

    def run(in_maps):
        concat_in = [np.concatenate([np.asarray(in_maps[c][n]) for c in range(NC)], axis=0)
                     for n in in_names]
        outs = jitted(*concat_in, *zeros_dev)
        outs = [np.asarray(o) for o in outs]
        results = []
        for c in range(NC):
            d = {}
            for name, o in zip(out_names, outs):
                per = o.shape[0] // NC
                d[name] = o[c * per:(c + 1) * per]
            results.append(d)
        return results

    return run


def kernel(**inputs):
    in_maps, flags = _prep_inputs(**inputs)
    if flags not in _NC_CACHE:
        nc = _build(*flags)
        _NC_CACHE[flags] = _make_runner(nc)
    run = _NC_CACHE[flags]
    res = run(in_maps)
    logits = np.concatenate([res[c]["logits_o"] for c in range(NC)], axis=2)
    attn = np.concatenate([res[c]["attn_o"] for c in range(NC)], axis=0)
    return logits, attn
